# revision 22
# baseline (speedup 1.0000x reference)
"""Trainium2 Bass kernel for AdaptiveLogSoftmaxWithLoss (moe_routing).

Sharding: the three class dimensions are zero-padded and tensor-sharded
across the 8 cores (head 4002->4096, tail0 16000 exact, tail1 30257->30720),
so every core runs an identical SPMD program with 1/8 of the output classes.

Row permutation: the reference discards a tail cluster's logsumexp for rows
whose target is not in that cluster, so the host permutes the sample axis to
[cluster1 rows | cluster2 rows | shortlist rows].  The head runs over all 16
sample tiles (order is irrelevant per-row); the tail pipelines only run over
the ~6 tiles holding cluster-1 rows and the ~10 tiles holding cluster-2 rows
(boundary tiles compute a few wasted rows that the host ignores).  This cuts
exp work ~40% and tail GEMM work ~45%, and the tails read the (permuted)
inpT directly - no compacted input copies.

Per core:
  - hidden projections first (their DMA lands earliest; h0T/h1T in bf16 for
    the XBAR target-dot transposes, plus fp8 copies as tail GEMM lhsT),
  - head logits (fp8 DoubleRow GEMMs, inp 16x / weights 64x) with tail0
    groups interleaved so ACT (the exp bottleneck) never starves: each
    PSUM group gets one ACT exp (+accum_out row sum-exp, descaled via the
    activation scale); logits are small (|x| < ~4) so no max subtraction,
  - head target logit via a fused DVE (iota==rel)*logit pass on the f32
    PSUM group (iota generated once on the idle GpSimd); tail target logits
    dot bf16 natural-layout hidden rows against host-gathered target weight
    rows zeroed on non-owner cores (all-bf16 streams hit the 2x DVE mode -
    an all-fp16 iota==rel pass on the exp output measured 1x and made DVE
    the bottleneck),
  - tail0 tiles split into two 2-bank PSUM groups (1024+976) so the head's
    1-bank pool and tail0's pool coexist; tail1 splits 3840 into a 2048-col
    A group (ACT accum) and 1792-col B group (DVE reduce).

Host combine: sum partials over cores, subtract the exact exp(0)=1
contribution of the zero-padded columns, lse = log(sum), then
NLL = -(head + masked tail terms) scattered back through the permutation.
"""

import numpy as np
import ml_dtypes

import concourse.bass as bass
import concourse.bacc as bacc
import concourse.mybir as mybir
import concourse.tile as tile
from concourse.bass_utils import run_bass_kernel_spmd

BF16 = ml_dtypes.bfloat16
FP8 = ml_dtypes.float8_e4m3
H_SCALE = 8.0     # h cast to fp8 at 8x
W_SCALE = 64.0    # tail w2 cast to fp8 at 64x
IN_SCALE = 16.0   # inp cast to fp8 at 16x
W1_SCALE = 64.0   # w1 / head_w cast to fp8 at 64x
HID_DESCALE = 1.0 / (IN_SCALE * W1_SCALE)
NCORES = 8
N, D = 2048, 1024
H0, H1 = 512, 256
C0, C1 = 4000, 20000
HEAD = 4002        # 4000 shortlist + 2 cluster-logit columns
HEAD_PAD = 4096    # padded so 8 cores get 512 each
T0 = 16000         # divides by 8 exactly (2000 each, no padding)
T1 = 30257
T1_PAD = 30720     # padded so 8 cores get 3840 each
WH, W0, W1 = HEAD_PAD // 8, T0 // 8, T1_PAD // 8     # 512, 2000, 3840
MT = N // 128                                        # 16 sample tiles
PAD_H = HEAD_PAD - HEAD   # 94 zero columns, all on core 7
PAD_1 = T1_PAD - T1       # 463 zero columns, all on core 7
NOREL = -1.0e9            # sentinel rel for rows outside the cluster

# module-level knobs for test.py (harness never touches these)
TRACE = False
LAST_RESULT = None

_CACHED_NC = {}


def _chunks(total, step=512):
    out, o = [], 0
    while o < total:
        out.append((o, min(step, total - o)))
        o += step
    return out


def _build_nc(t0e, b2, e2):
    # tail0 runs on sample tiles [0, t0e); tail1 on [b2, e2)
    n2t = e2 - b2
    n0p, n2p = t0e * 128, n2t * 128
    NM = MT + t0e + n2t
    nc = bacc.Bacc(None)
    BF = mybir.dt.bfloat16
    F8 = mybir.dt.float8e4
    F32 = mybir.dt.float32
    AX = mybir.AxisListType
    OP = mybir.AluOpType
    ACTF = mybir.ActivationFunctionType

    inpT_d = nc.dram_tensor("inpT", [128, D // 128, N], F8, kind="ExternalInput")
    inpT1_d = nc.dram_tensor("inpT1", [128, D // 128, max(n0p, 128)], F8,
                             kind="ExternalInput")
    hwT_d = nc.dram_tensor("hwT", [128, D // 128, WH], F8, kind="ExternalInput")
    w1t0_d = nc.dram_tensor("w1t0", [128, D // 128, H0], F8, kind="ExternalInput")
    w1t1_d = nc.dram_tensor("w1t1", [128, D // 128, H1], F8, kind="ExternalInput")
    w2t0_d = nc.dram_tensor("w2t0", [128, H0 // 128, W0], F8, kind="ExternalInput")
    w2t1_d = nc.dram_tensor("w2t1", [128, H1 // 128, W1], F8, kind="ExternalInput")
    wg0_d = nc.dram_tensor("wg0", [128, max(t0e, 1), H0], BF, kind="ExternalInput")
    wg1_d = nc.dram_tensor("wg1", [128, max(n2t, 1), H1], BF, kind="ExternalInput")
    misc_d = nc.dram_tensor("misc", [128, MT], F32, kind="ExternalInput")
    scr_d = nc.dram_tensor("scr", [128, 512], F8, kind="ExternalOutput")
    res_d = nc.dram_tensor("res", [128, NM, 4], F32, kind="ExternalOutput")

    with tile.TileContext(nc) as tc:
        with (
            tc.tile_pool(name="const", bufs=1) as cp,
            tc.tile_pool(name="work", bufs=3) as wp,
        ):
            inpT = cp.tile([128, D // 128, N], F8)
            inpT1 = cp.tile([128, D // 128, max(n0p, 128)], F8)
            hwT = cp.tile([128, D // 128, WH], F8)
            w1t0 = cp.tile([128, D // 128, H0], F8)
            w1t1 = cp.tile([128, D // 128, H1], F8)
            w2t0 = cp.tile([128, H0 // 128, W0], F8)
            w2t1 = cp.tile([128, H1 // 128, W1], F8)
            wg0 = cp.tile([128, max(t0e, 1), H0], BF)
            wg1 = cp.tile([128, max(n2t, 1), H1], BF)
            relH = cp.tile([128, MT], F32)
            iota = cp.tile([128, WH], F32)
            h0T = cp.tile([128, H0 // 128, max(n0p, 128)], BF)
            h1T = cp.tile([128, H1 // 128, max(n2p, 128)], BF)
            h0T8 = cp.tile([128, H0 // 128, max(n0p, 128)], F8)
            h1T8 = cp.tile([128, H1 // 128, max(n2p, 128)], F8)
            h0n = cp.tile([128, max(t0e, 1), H0], BF)
            h1n = cp.tile([128, max(n2t, 1), H1], BF)
            res = cp.tile([128, NM, 4], F32)

            zz = wp.tile([128, 2], F32, tag="zz")
            nc.vector.memset(zz[:, 0:1], 0)
            nc.scalar.activation(zz[:, 1:2], zz[:, 0:1], ACTF.Exp)
            nc.gpsimd.memset(res[:], 0)
            nc.gpsimd.iota(
                iota[:],
                pattern=[[1, WH]],
                base=0,
                channel_multiplier=0,
                allow_small_or_imprecise_dtypes=True,
            )

            # loads staged in waves: the SDMA engines round-robin across
            # queued transfers, so issuing everything at once starves the
            # critical early tensors.  Later waves sit behind a dummy store
            # whose data dep releases them once hidden0 has progressed.
            nc.sync.dma_start(w1t0[:], w1t0_d[:])
            if t0e:
                nc.sync.dma_start(inpT1[:], inpT1_d[:])

            # head + hidden use 4 one-bank slots, tail0 halves 2 two-bank
            # slots (8 banks total); both close before tail1's 2 four-bank
            # slots open (the boundary is data-serialized on h1T8).
            fpool_cm = tc.tile_pool(name="psumF", bufs=4, space="PSUM")
            fpool = fpool_cm.__enter__()
            mpool_cm = tc.tile_pool(name="psumM", bufs=2, space="PSUM")
            mpool = mpool_cm.__enter__()
            psp = None

            def fslot(w):
                ps = fpool.tile([128, 512], F32, tag="front", name="ps")
                return ps[:, :w]

            def mslot(w):
                ps = mpool.tile([128, 1024], F32, tag="mid", name="ps")
                return ps[:, :w]

            def pslot(w):
                ps = psp.tile([128, 2048], F32, tag="logits", name="ps")
                return ps[:, :w]

            DESCALE = 1.0 / (H_SCALE * W_SCALE)
            DR = mybir.MatmulPerfMode.DoubleRow

            def hidden_block(hT, hT8, w1, inT, coff, npad, mh, alloc):
                # one h k-tile: [128 h, npad samples] in <=512-col chunks
                for co, cw in _chunks(npad):
                    ps = alloc(cw)
                    for kt in range(0, D // 128, 2):
                        nc.tensor.matmul(
                            ps[:],
                            w1[:, kt : kt + 2, mh * 128 : (mh + 1) * 128],
                            inT[:, kt : kt + 2, coff + co : coff + co + cw],
                            start=(kt == 0),
                            stop=(kt + 2 >= D // 128),
                            perf_mode=DR,
                        )
                    nc.vector.tensor_scalar_mul(
                        hT[:, mh, co : co + cw], ps[:], HID_DESCALE
                    )
                    nc.vector.tensor_scalar_mul(
                        hT8[:, mh, co : co + cw], hT[:, mh, co : co + cw],
                        H_SCALE,
                    )

            def head_group(m):
                ms = slice(m * 128, (m + 1) * 128)
                ps = fslot(WH)
                for kt in range(0, D // 128, 2):
                    nc.tensor.matmul(
                        ps[:],
                        inpT[:, kt : kt + 2, ms],
                        hwT[:, kt : kt + 2, :],
                        start=(kt == 0),
                        stop=(kt + 2 >= D // 128),
                        perf_mode=DR,
                    )
                sc_e = wp.tile([128, 2048], BF, tag="sc_e")
                nc.scalar.activation(
                    sc_e[:, :WH],
                    ps[:],
                    ACTF.Exp,
                    scale=HID_DESCALE,
                    accum_out=res[:, m, 0:1],
                )
                sc_t = wp.tile([128, WH], BF, tag="sc_td")
                nc.vector.scalar_tensor_tensor(
                    out=sc_t[:],
                    in0=iota[:],
                    scalar=relH[:, m : m + 1],
                    in1=ps[:],
                    op0=OP.is_equal,
                    op1=OP.mult,
                    accum_out=res[:, m, 1:2],
                )

            def tail_group(lhsT, w2, kdim, mt, gw, goff, s_ap, alloc):
                # fp8 DoubleRow GEMM group + exp/accum partial sum
                ms = slice(mt * 128, (mt + 1) * 128)
                ps = alloc(gw)
                nsub = kdim // 128
                for co, cw in _chunks(gw):
                    for kt in range(0, nsub, 2):
                        nc.tensor.matmul(
                            ps[:, co : co + cw],
                            lhsT[:, kt : kt + 2, ms],
                            w2[:, kt : kt + 2, goff + co : goff + co + cw],
                            start=(kt == 0),
                            stop=(kt + 2 >= nsub),
                            perf_mode=DR,
                        )
                sc_e = wp.tile([128, 2048], BF, tag="sc_e")
                nc.scalar.activation(
                    sc_e[:, :gw], ps[:], ACTF.Exp, scale=DESCALE,
                    accum_out=s_ap,
                )
                return sc_e

            def transposes(hT, hn, hdim):
                # batched XBAR transpose hT[h, r] -> hn[r, h]:
                # out[p, j, q] = in[q, j*128+p]
                for kt in range(hdim // 128):
                    nc.sync.dma_start_transpose(
                        hn[:, :, kt * 128 : (kt + 1) * 128], hT[:, kt, :]
                    )

            def dot(hn, wg, hdim, mt, t_ap):
                sc_d = wp.tile([128, H0], BF, tag="sc_td")
                nc.vector.scalar_tensor_tensor(
                    out=sc_d[:, :hdim],
                    in0=hn[:, mt, :],
                    scalar=1.0,
                    in1=wg[:, mt, :],
                    op0=OP.mult,
                    op1=OP.mult,
                    accum_out=t_ap,
                )

            def tail0_tile(mt):
                tail_group(h0T8, w2t0, H0, mt, 1024, 0,
                           res[:, MT + mt, 0:1], mslot)
                tail_group(h0T8, w2t0, H0, mt, W0 - 1024, 1024,
                           res[:, MT + mt, 2:3], mslot)
                dot(h0n, wg0, H0, mt, res[:, MT + mt, 1:2])

            # emission: hidden0 -> head m0,m1 -> {tail0 t, head m} pairs ->
            # hidden1 woven into the remaining head groups -> tail1
            if True:
                if t0e:
                    hidden_block(h0T, h0T8, w1t0, inpT1, 0, n0p, 0, fslot)
                    nc.sync.dma_start(scr_d[:, 0:128], h0T8[:, 0, 0:128])
                nc.sync.dma_start(hwT[:], hwT_d[:])
                nc.sync.dma_start(inpT[:, :, 0:1024], inpT_d[:, :, 0:1024])
                if t0e:
                    hidden_block(h0T, h0T8, w1t0, inpT1, 0, n0p, 1, fslot)
                    nc.sync.dma_start(scr_d[:, 128:256], h0T8[:, 1, 0:128])
                nc.sync.dma_start(inpT[:, :, 1024:2048], inpT_d[:, :, 1024:2048])
                nc.sync.dma_start(relH[:], misc_d[:])
                if t0e:
                    for mh in range(2, H0 // 128):
                        hidden_block(h0T, h0T8, w1t0, inpT1, 0, n0p, mh, fslot)
                    nc.sync.dma_start(scr_d[:, 384:512], h0T8[:, 3, 0:128])
                nc.sync.dma_start(w2t0[:], w2t0_d[:])
                if t0e:
                    nc.sync.dma_start(wg0[:], wg0_d[:])
                nc.sync.dma_start(w1t1[:], w1t1_d[:])
                if t0e:
                    transposes(h0T, h0n, H0)

            if True:
                m = 0
                while m < min(6, MT):
                    head_group(m)
                    m += 1
                for mt in range(max(t0e - 2, 0)):
                    tail0_tile(mt)
                    if m < MT:
                        head_group(m)
                        m += 1
                h1_at = {max(m, MT - 5): 0, MT - 3: 1} if n2t else {}
                while m < MT:
                    if m in h1_at:
                        hidden_block(
                            h1T, h1T8, w1t1, inpT, b2 * 128, n2p, h1_at[m],
                            fslot,
                        )
                    head_group(m)
                    m += 1
                if t0e >= 2:
                    tail0_tile(t0e - 2)
                if t0e:
                    tail0_tile(t0e - 1)
                nc.sync.dma_start(res_d[:, 0:MT], res[:, 0:MT])
                if t0e:
                    nc.sync.dma_start(
                        res_d[:, MT : MT + t0e], res[:, MT : MT + t0e]
                    )
                if n2t:
                    # ensure both hidden1 k-tiles were emitted
                    emitted = {h1_at[k] for k in h1_at if k < MT}
                    for mh in (0, 1):
                        if mh not in emitted:
                            hidden_block(
                                h1T, h1T8, w1t1, inpT, b2 * 128, n2p, mh,
                                fslot,
                            )
                    nc.sync.dma_start(scr_d[:, 256:384], h1T8[:, 1, 0:128])
                nc.sync.dma_start(w2t1[:], w2t1_d[:])
                if n2t:
                    nc.sync.dma_start(wg1[:], wg1_d[:])
                    transposes(h1T, h1n, H1)
            mpool_cm.__exit__(None, None, None)
            fpool_cm.__exit__(None, None, None)
            psp_cm = tc.tile_pool(name="psum", bufs=2, space="PSUM")
            psp = psp_cm.__enter__()
            if True:
                BW = W1 - 2048  # 1792-wide B group
                for mt in range(n2t):
                    ri = MT + t0e + mt
                    dot(h1n, wg1, H1, mt, res[:, ri, 1:2])
                    # B group first: exp on ACT without accum, sum on DVE,
                    # so the final ACT exp (A group) overlaps the B reduce
                    ms = slice(mt * 128, (mt + 1) * 128)
                    ps = pslot(BW)
                    for co, cw in _chunks(BW):
                        nc.tensor.matmul(
                            ps[:, co : co + cw],
                            h1T8[:, 0:2, ms],
                            w2t1[:, 0:2, 2048 + co : 2048 + co + cw],
                            start=True,
                            stop=True,
                            perf_mode=DR,
                        )
                    sc_b = wp.tile([128, 2048], BF, tag="sc_e")
                    nc.scalar.activation(
                        sc_b[:, :BW], ps[:], ACTF.Exp, scale=DESCALE
                    )
                    nc.vector.reduce_sum(res[:, ri, 2:3], sc_b[:, :BW], axis=AX.X)
                    tail_group(h1T8, w2t1, H1, mt, 2048, 0,
                               res[:, ri, 0:1], pslot)

            psp_cm.__exit__(None, None, None)
            if n2t:
                nc.sync.dma_start(
                    res_d[:, MT + t0e :], res[:, MT + t0e :]
                )

    nc.finalize()
    return nc


def _get_nc(t0e, b2, e2):
    key = (t0e, b2, e2)
    if key not in _CACHED_NC:
        _CACHED_NC[key] = _build_nc(t0e, b2, e2)
    return _CACHED_NC[key]


def _tiled(a2d):
    """[K, F] (K multiple of 128) -> contiguous [128, K//128, F]."""
    K, F = a2d.shape
    return np.ascontiguousarray(
        a2d.reshape(K // 128, 128, F).transpose(1, 0, 2)
    )


def _pm(vec, nt):
    """[nt*128] -> [128, nt] with [p, m] = vec[m*128+p]."""
    return np.ascontiguousarray(vec.reshape(nt, 128).T.astype(np.float32))


def make_in_maps(inp, tgt, head_w, t0_w1, t0_w2, t1_w1, t1_w2):
    inp = np.asarray(inp, dtype=np.float32)
    tgt = np.asarray(tgt).astype(np.int64)

    in1 = (tgt >= C0) & (tgt < C1)
    in2 = tgt >= C1
    perm = np.concatenate(
        [np.nonzero(in1)[0], np.nonzero(in2)[0], np.nonzero(~(in1 | in2))[0]]
    )
    n1 = int(in1.sum())
    n2 = int(in2.sum())
    t0e = (n1 + 127) // 128
    b2 = n1 // 128
    e2 = (n1 + n2 + 127) // 128 if n2 else b2
    n2t = e2 - b2

    tgt_p = tgt[perm]
    inpq = (inp[perm].T * IN_SCALE).astype(FP8)
    inpT = _tiled(inpq)
    if t0e:
        c = np.zeros((D, max(t0e * 128, 128)), FP8)
        c[:, : t0e * 128] = inpq[:, : t0e * 128]
        inpT1 = _tiled(c)

    w1t0 = _tiled((np.asarray(t0_w1, np.float32).T * W1_SCALE).astype(FP8))
    w1t1 = _tiled((np.asarray(t1_w1, np.float32).T * W1_SCALE).astype(FP8))

    hwT_full = np.zeros((D, HEAD_PAD), FP8)
    hwT_full[:, :HEAD] = (np.asarray(head_w, np.float32).T * W1_SCALE).astype(FP8)
    w2t0_full = (np.asarray(t0_w2, np.float32).T * W_SCALE).astype(FP8)
    w2t1_full = np.zeros((H1, T1_PAD), FP8)
    w2t1_full[:, :T1] = (np.asarray(t1_w2, np.float32).T * W_SCALE).astype(FP8)

    gi = np.where(tgt_p < C0, tgt_p, np.where(tgt_p < C1, C0, C0 + 1))
    rel0 = np.where(
        (tgt_p >= C0) & (tgt_p < C1), tgt_p - C0, np.int64(NOREL)
    )[: t0e * 128]
    rel1 = np.where(tgt_p >= C1, tgt_p - C1, np.int64(NOREL))[
        b2 * 128 : e2 * 128
    ]

    # host-gathered target weight rows (bf16, matching device operand
    # precision), zeroed on cores that don't own the target's column shard
    t0_w2_bf = np.asarray(t0_w2, np.float32).astype(BF16)
    t1_w2_bf = np.asarray(t1_w2, np.float32).astype(BF16)

    def _gather_rows(tbl, row, own, nt):
        g = tbl[np.clip(row, 0, tbl.shape[0] - 1)]
        g[~own] = 0
        return np.ascontiguousarray(
            g.reshape(nt, 128, tbl.shape[1]).transpose(1, 0, 2)
        )

    in_maps = []
    for i in range(NCORES):
        m = {
            "inpT": inpT,
            **({"inpT1": inpT1} if t0e else {}),
            "hwT": _tiled(hwT_full[:, i * WH : (i + 1) * WH]),
            "w1t0": w1t0,
            "w1t1": w1t1,
            "w2t0": _tiled(w2t0_full[:, i * W0 : (i + 1) * W0]),
            "w2t1": _tiled(w2t1_full[:, i * W1 : (i + 1) * W1]),
            "misc": _pm((gi - i * WH).astype(np.float64), MT),
        }
        if t0e:
            m["wg0"] = _gather_rows(
                t0_w2_bf, rel0, ((rel0 // W0) == i) & (rel0 >= 0), t0e
            )
        if n2t:
            m["wg1"] = _gather_rows(
                t1_w2_bf, rel1, ((rel1 // W1) == i) & (rel1 >= 0), n2t
            )
        in_maps.append(m)
    return in_maps, perm, n1, n2, t0e, b2, e2


def combine(results, perm, n1, n2, t0e, b2, e2):
    """per-core {'res': [128, NM, 4]} partials -> final [N] f32 NLL."""
    n2t = e2 - b2
    NM = MT + t0e + n2t
    acc = np.zeros((128, NM, 4), np.float64)
    for r in results:
        acc += np.asarray(r["res"], np.float64)

    def col(c):  # [128, NM] -> [NM*128] with row = m*128+p
        return acc[:, :, c].T.reshape(-1)

    S, T, SB = col(0), col(1), col(2)

    S_head = S[:N] - PAD_H  # zero-padded cols contribute exp(0)=1 (core 7)
    out = T[:N] * HID_DESCALE - np.log(S_head)

    if t0e:
        s = slice(N, N + t0e * 128)
        lp0 = T[s] - np.log(S[s] + SB[s])
        out[:n1] += lp0[:n1]
    if n2t:
        s = slice(N + t0e * 128, N + (t0e + n2t) * 128)
        lp1 = T[s] - np.log(S[s] + SB[s] - PAD_1)
        lo = b2 * 128
        out[n1 : n1 + n2] += lp1[n1 - lo : n1 - lo + n2]

    res = np.empty(N, np.float32)
    res[perm] = (-out).astype(np.float32)
    return res


def kernel(inp, tgt, head_w, t0_w1, t0_w2, t1_w1, t1_w2):
    global LAST_RESULT
    in_maps, perm, n1, n2, t0e, b2, e2 = make_in_maps(
        inp, tgt, head_w, t0_w1, t0_w2, t1_w1, t1_w2
    )
    nc = _get_nc(t0e, b2, e2)
    out = run_bass_kernel_spmd(
        nc, in_maps, core_ids=list(range(NCORES)), trace=TRACE
    )
    LAST_RESULT = out
    return combine(out.results, perm, n1, n2, t0e, b2, e2)


# revision 23
# speedup vs baseline: 1.0193x; 1.0193x over previous
"""Trainium2 Bass kernel for AdaptiveLogSoftmaxWithLoss (moe_routing).

Sharding: the three class dimensions are zero-padded and tensor-sharded
across the 8 cores (head 4002->4096, tail0 16000 exact, tail1 30257->30720),
so every core runs an identical SPMD program with 1/8 of the output classes.

Row permutation: the reference discards a tail cluster's logsumexp for rows
whose target is not in that cluster, so the host permutes the sample axis to
[cluster1 rows | cluster2 rows | shortlist rows].  The head runs over all 16
sample tiles (order is irrelevant per-row); the tail pipelines only run over
the ~6 tiles holding cluster-1 rows and the ~10 tiles holding cluster-2 rows
(boundary tiles compute a few wasted rows that the host ignores).  This cuts
exp work ~40% and tail GEMM work ~45%, and the tails read the (permuted)
inpT directly - no compacted input copies.

Per core:
  - hidden projections first (their DMA lands earliest; h0T/h1T in bf16 for
    the XBAR target-dot transposes, plus fp8 copies as tail GEMM lhsT),
  - head logits (fp8 DoubleRow GEMMs, inp 16x / weights 64x) with tail0
    groups interleaved so ACT (the exp bottleneck) never starves: each
    PSUM group gets one ACT exp (+accum_out row sum-exp, descaled via the
    activation scale); logits are small (|x| < ~4) so no max subtraction,
  - head target logit via a fused DVE (iota==rel)*logit pass on the f32
    PSUM group (iota generated once on the idle GpSimd); tail target logits
    dot bf16 natural-layout hidden rows against host-gathered target weight
    rows zeroed on non-owner cores (all-bf16 streams hit the 2x DVE mode -
    an all-fp16 iota==rel pass on the exp output measured 1x and made DVE
    the bottleneck),
  - tail0 tiles split into two 2-bank PSUM groups (1024+976) so the head's
    1-bank pool and tail0's pool coexist; tail1 splits 3840 into a 2048-col
    A group (ACT accum) and 1792-col B group (DVE reduce).

Host combine: sum partials over cores, subtract the exact exp(0)=1
contribution of the zero-padded columns, lse = log(sum), then
NLL = -(head + masked tail terms) scattered back through the permutation.
"""

import numpy as np
import ml_dtypes

import concourse.bass as bass
import concourse.bacc as bacc
import concourse.mybir as mybir
import concourse.tile as tile
from concourse.bass_utils import run_bass_kernel_spmd

BF16 = ml_dtypes.bfloat16
FP8 = ml_dtypes.float8_e4m3
H_SCALE = 8.0     # h cast to fp8 at 8x
W_SCALE = 64.0    # tail w2 cast to fp8 at 64x
IN_SCALE = 16.0   # inp cast to fp8 at 16x
W1_SCALE = 64.0   # w1 / head_w cast to fp8 at 64x
HID_DESCALE = 1.0 / (IN_SCALE * W1_SCALE)
NCORES = 8
N, D = 2048, 1024
H0, H1 = 512, 256
C0, C1 = 4000, 20000
HEAD = 4002        # 4000 shortlist + 2 cluster-logit columns
HEAD_PAD = 4096    # padded so 8 cores get 512 each
T0 = 16000         # divides by 8 exactly (2000 each, no padding)
T1 = 30257
T1_PAD = 30720     # padded so 8 cores get 3840 each
WH, W0, W1 = HEAD_PAD // 8, T0 // 8, T1_PAD // 8     # 512, 2000, 3840
MT = N // 128                                        # 16 sample tiles
PAD_H = HEAD_PAD - HEAD   # 94 zero columns, all on core 7
PAD_1 = T1_PAD - T1       # 463 zero columns, all on core 7
NOREL = -1.0e9            # sentinel rel for rows outside the cluster

# module-level knobs for test.py (harness never touches these)
TRACE = False
LAST_RESULT = None

_CACHED_NC = {}


def _chunks(total, step=512):
    out, o = [], 0
    while o < total:
        out.append((o, min(step, total - o)))
        o += step
    return out


def _build_nc(t0e, b2, e2):
    # tail0 runs on sample tiles [0, t0e); tail1 on [b2, e2)
    n2t = e2 - b2
    n0p, n2p = t0e * 128, n2t * 128
    NM = MT + t0e + n2t
    nc = bacc.Bacc(None)
    BF = mybir.dt.bfloat16
    F8 = mybir.dt.float8e4
    F32 = mybir.dt.float32
    AX = mybir.AxisListType
    OP = mybir.AluOpType
    ACTF = mybir.ActivationFunctionType

    inpT_d = nc.dram_tensor("inpT", [128, D // 128, N], F8, kind="ExternalInput")
    inpT1_d = nc.dram_tensor("inpT1", [128, D // 128, max(n0p, 128)], F8,
                             kind="ExternalInput")
    hwT_d = nc.dram_tensor("hwT", [128, D // 128, WH], F8, kind="ExternalInput")
    w1t0_d = nc.dram_tensor("w1t0", [128, D // 128, H0], F8, kind="ExternalInput")
    w1t1_d = nc.dram_tensor("w1t1", [128, D // 128, H1], F8, kind="ExternalInput")
    w2t0_d = nc.dram_tensor("w2t0", [128, H0 // 128, W0], F8, kind="ExternalInput")
    w2t1_d = nc.dram_tensor("w2t1", [128, H1 // 128, W1], F8, kind="ExternalInput")
    wg0_d = nc.dram_tensor("wg0", [128, max(t0e, 1), H0], BF, kind="ExternalInput")
    wg1_d = nc.dram_tensor("wg1", [128, max(n2t, 1), H1], BF, kind="ExternalInput")
    misc_d = nc.dram_tensor("misc", [128, MT], F32, kind="ExternalInput")
    scr_d = nc.dram_tensor("scr", [128, 512], F8, kind="ExternalOutput")
    res_d = nc.dram_tensor("res", [128, NM, 4], F32, kind="ExternalOutput")

    with tile.TileContext(nc) as tc:
        with (
            tc.tile_pool(name="const", bufs=1) as cp,
            tc.tile_pool(name="work", bufs=4) as wp,
        ):
            inpT = cp.tile([128, D // 128, N], F8)
            inpT1 = cp.tile([128, D // 128, max(n0p, 128)], F8)
            hwT = cp.tile([128, D // 128, WH], F8)
            w1t0 = cp.tile([128, D // 128, H0], F8)
            w1t1 = cp.tile([128, D // 128, H1], F8)
            w2t0 = cp.tile([128, H0 // 128, W0], F8)
            w2t1 = cp.tile([128, H1 // 128, W1], F8)
            wg0 = cp.tile([128, max(t0e, 1), H0], BF)
            wg1 = cp.tile([128, max(n2t, 1), H1], BF)
            relH = cp.tile([128, MT], F32)
            iota = cp.tile([128, WH], F32)
            h0T = cp.tile([128, H0 // 128, max(n0p, 128)], BF)
            h1T = cp.tile([128, H1 // 128, max(n2p, 128)], BF)
            h0T8 = cp.tile([128, H0 // 128, max(n0p, 128)], F8)
            h1T8 = cp.tile([128, H1 // 128, max(n2p, 128)], F8)
            h0n = cp.tile([128, max(t0e, 1), H0], BF)
            h1n = cp.tile([128, max(n2t, 1), H1], BF)
            res = cp.tile([128, NM, 4], F32)

            zz = wp.tile([128, 2], F32, tag="zz")
            nc.vector.memset(zz[:, 0:1], 0)
            nc.scalar.activation(zz[:, 1:2], zz[:, 0:1], ACTF.Exp)
            nc.gpsimd.memset(res[:], 0)
            nc.gpsimd.iota(
                iota[:],
                pattern=[[1, WH]],
                base=0,
                channel_multiplier=0,
                allow_small_or_imprecise_dtypes=True,
            )

            # loads staged in waves: the SDMA engines round-robin across
            # queued transfers, so issuing everything at once starves the
            # critical early tensors.  Later waves sit behind a dummy store
            # whose data dep releases them once hidden0 has progressed.
            nc.sync.dma_start(w1t0[:], w1t0_d[:])
            if t0e:
                nc.sync.dma_start(inpT1[:], inpT1_d[:])

            # head + hidden use 4 one-bank slots, tail0 halves 2 two-bank
            # slots (8 banks total); both close before tail1's 2 four-bank
            # slots open (the boundary is data-serialized on h1T8).
            fpool_cm = tc.tile_pool(name="psumF", bufs=4, space="PSUM")
            fpool = fpool_cm.__enter__()
            mpool_cm = tc.tile_pool(name="psumM", bufs=2, space="PSUM")
            mpool = mpool_cm.__enter__()
            psp = None

            def fslot(w):
                ps = fpool.tile([128, 512], F32, tag="front", name="ps")
                return ps[:, :w]

            def mslot(w):
                ps = mpool.tile([128, 1024], F32, tag="mid", name="ps")
                return ps[:, :w]

            def pslot(w):
                ps = psp.tile([128, 2048], F32, tag="logits", name="ps")
                return ps[:, :w]

            DESCALE = 1.0 / (H_SCALE * W_SCALE)
            DR = mybir.MatmulPerfMode.DoubleRow

            def hidden_block(hT, hT8, w1, inT, coff, npad, mh, alloc):
                # one h k-tile: [128 h, npad samples] in <=512-col chunks
                for co, cw in _chunks(npad):
                    ps = alloc(cw)
                    for kt in range(0, D // 128, 2):
                        nc.tensor.matmul(
                            ps[:],
                            w1[:, kt : kt + 2, mh * 128 : (mh + 1) * 128],
                            inT[:, kt : kt + 2, coff + co : coff + co + cw],
                            start=(kt == 0),
                            stop=(kt + 2 >= D // 128),
                            perf_mode=DR,
                        )
                    nc.vector.tensor_scalar_mul(
                        hT[:, mh, co : co + cw], ps[:], HID_DESCALE
                    )
                    nc.vector.tensor_scalar_mul(
                        hT8[:, mh, co : co + cw], hT[:, mh, co : co + cw],
                        H_SCALE,
                    )

            def head_group(m):
                ms = slice(m * 128, (m + 1) * 128)
                ps = fslot(WH)
                for kt in range(0, D // 128, 2):
                    nc.tensor.matmul(
                        ps[:],
                        inpT[:, kt : kt + 2, ms],
                        hwT[:, kt : kt + 2, :],
                        start=(kt == 0),
                        stop=(kt + 2 >= D // 128),
                        perf_mode=DR,
                    )
                sc_e = wp.tile([128, 2048], BF, tag="sc_e")
                nc.scalar.activation(
                    sc_e[:, :WH],
                    ps[:],
                    ACTF.Exp,
                    scale=HID_DESCALE,
                    accum_out=res[:, m, 0:1],
                )
                sc_t = wp.tile([128, WH], BF, tag="sc_td")
                nc.vector.scalar_tensor_tensor(
                    out=sc_t[:],
                    in0=iota[:],
                    scalar=relH[:, m : m + 1],
                    in1=ps[:],
                    op0=OP.is_equal,
                    op1=OP.mult,
                    accum_out=res[:, m, 1:2],
                )

            def tail_group(lhsT, w2, kdim, mt, gw, goff, s_ap, alloc):
                # fp8 DoubleRow GEMM group + exp/accum partial sum
                ms = slice(mt * 128, (mt + 1) * 128)
                ps = alloc(gw)
                nsub = kdim // 128
                for co, cw in _chunks(gw):
                    for kt in range(0, nsub, 2):
                        nc.tensor.matmul(
                            ps[:, co : co + cw],
                            lhsT[:, kt : kt + 2, ms],
                            w2[:, kt : kt + 2, goff + co : goff + co + cw],
                            start=(kt == 0),
                            stop=(kt + 2 >= nsub),
                            perf_mode=DR,
                        )
                sc_e = wp.tile([128, 2048], BF, tag="sc_e")
                nc.scalar.activation(
                    sc_e[:, :gw], ps[:], ACTF.Exp, scale=DESCALE,
                    accum_out=s_ap,
                )
                return sc_e

            def transposes(hT, hn, hdim):
                # batched XBAR transpose hT[h, r] -> hn[r, h]:
                # out[p, j, q] = in[q, j*128+p]
                for kt in range(hdim // 128):
                    nc.sync.dma_start_transpose(
                        hn[:, :, kt * 128 : (kt + 1) * 128], hT[:, kt, :]
                    )

            def dot(hn, wg, hdim, mt, t_ap):
                sc_d = wp.tile([128, H0], BF, tag="sc_td")
                nc.vector.scalar_tensor_tensor(
                    out=sc_d[:, :hdim],
                    in0=hn[:, mt, :],
                    scalar=1.0,
                    in1=wg[:, mt, :],
                    op0=OP.mult,
                    op1=OP.mult,
                    accum_out=t_ap,
                )

            def tail0_tile(mt):
                tail_group(h0T8, w2t0, H0, mt, 1024, 0,
                           res[:, MT + mt, 0:1], mslot)
                tail_group(h0T8, w2t0, H0, mt, W0 - 1024, 1024,
                           res[:, MT + mt, 2:3], mslot)
                dot(h0n, wg0, H0, mt, res[:, MT + mt, 1:2])

            # emission: hidden0 -> head m0,m1 -> {tail0 t, head m} pairs ->
            # hidden1 woven into the remaining head groups -> tail1
            if True:
                if t0e:
                    hidden_block(h0T, h0T8, w1t0, inpT1, 0, n0p, 0, fslot)
                    nc.sync.dma_start(scr_d[:, 0:128], h0T8[:, 0, 0:128])
                nc.sync.dma_start(hwT[:], hwT_d[:])
                nc.sync.dma_start(inpT[:, :, 0:1024], inpT_d[:, :, 0:1024])
                if t0e:
                    hidden_block(h0T, h0T8, w1t0, inpT1, 0, n0p, 1, fslot)
                    nc.sync.dma_start(scr_d[:, 128:256], h0T8[:, 1, 0:128])
                nc.sync.dma_start(inpT[:, :, 1024:2048], inpT_d[:, :, 1024:2048])
                nc.sync.dma_start(relH[:], misc_d[:])
                if t0e:
                    for mh in range(2, H0 // 128):
                        hidden_block(h0T, h0T8, w1t0, inpT1, 0, n0p, mh, fslot)
                    nc.sync.dma_start(scr_d[:, 384:512], h0T8[:, 3, 0:128])
                nc.sync.dma_start(w2t0[:], w2t0_d[:])
                if t0e:
                    nc.sync.dma_start(wg0[:], wg0_d[:])
                nc.sync.dma_start(w1t1[:], w1t1_d[:])
                if t0e:
                    transposes(h0T, h0n, H0)

            if True:
                m = 0
                while m < min(6, MT):
                    head_group(m)
                    m += 1
                for mt in range(max(t0e - 1, 0)):
                    tail0_tile(mt)
                    if m < MT:
                        head_group(m)
                        m += 1
                h1_at = {max(m, MT - 5): 0, MT - 3: 1} if n2t else {}
                while m < MT:
                    if m in h1_at:
                        hidden_block(
                            h1T, h1T8, w1t1, inpT, b2 * 128, n2p, h1_at[m],
                            fslot,
                        )
                    head_group(m)
                    m += 1
                if t0e:
                    tail0_tile(t0e - 1)
                nc.sync.dma_start(res_d[:, 0:MT], res[:, 0:MT])
                if t0e:
                    nc.sync.dma_start(
                        res_d[:, MT : MT + t0e], res[:, MT : MT + t0e]
                    )
                if n2t:
                    # ensure both hidden1 k-tiles were emitted
                    emitted = {h1_at[k] for k in h1_at if k < MT}
                    for mh in (0, 1):
                        if mh not in emitted:
                            hidden_block(
                                h1T, h1T8, w1t1, inpT, b2 * 128, n2p, mh,
                                fslot,
                            )
                    nc.sync.dma_start(scr_d[:, 256:384], h1T8[:, 1, 0:128])
                nc.sync.dma_start(w2t1[:], w2t1_d[:])
                if n2t:
                    nc.sync.dma_start(wg1[:], wg1_d[:])
                    transposes(h1T, h1n, H1)
            mpool_cm.__exit__(None, None, None)
            fpool_cm.__exit__(None, None, None)
            psp_cm = tc.tile_pool(name="psum", bufs=2, space="PSUM")
            psp = psp_cm.__enter__()
            if True:
                BW = W1 - 2048  # 1792-wide B group
                for mt in range(n2t):
                    ri = MT + t0e + mt
                    dot(h1n, wg1, H1, mt, res[:, ri, 1:2])
                    # B group first: exp on ACT without accum, sum on DVE,
                    # so the final ACT exp (A group) overlaps the B reduce
                    ms = slice(mt * 128, (mt + 1) * 128)
                    ps = pslot(BW)
                    for co, cw in _chunks(BW):
                        nc.tensor.matmul(
                            ps[:, co : co + cw],
                            h1T8[:, 0:2, ms],
                            w2t1[:, 0:2, 2048 + co : 2048 + co + cw],
                            start=True,
                            stop=True,
                            perf_mode=DR,
                        )
                    sc_b = wp.tile([128, 2048], BF, tag="sc_e")
                    nc.scalar.activation(
                        sc_b[:, :BW], ps[:], ACTF.Exp, scale=DESCALE
                    )
                    nc.vector.reduce_sum(res[:, ri, 2:3], sc_b[:, :BW], axis=AX.X)
                    tail_group(h1T8, w2t1, H1, mt, 2048, 0,
                               res[:, ri, 0:1], pslot)

            psp_cm.__exit__(None, None, None)
            if n2t:
                nc.sync.dma_start(
                    res_d[:, MT + t0e :], res[:, MT + t0e :]
                )

    nc.finalize()
    return nc


def _get_nc(t0e, b2, e2):
    key = (t0e, b2, e2)
    if key not in _CACHED_NC:
        _CACHED_NC[key] = _build_nc(t0e, b2, e2)
    return _CACHED_NC[key]


def _tiled(a2d):
    """[K, F] (K multiple of 128) -> contiguous [128, K//128, F]."""
    K, F = a2d.shape
    return np.ascontiguousarray(
        a2d.reshape(K // 128, 128, F).transpose(1, 0, 2)
    )


def _pm(vec, nt):
    """[nt*128] -> [128, nt] with [p, m] = vec[m*128+p]."""
    return np.ascontiguousarray(vec.reshape(nt, 128).T.astype(np.float32))


def make_in_maps(inp, tgt, head_w, t0_w1, t0_w2, t1_w1, t1_w2):
    inp = np.asarray(inp, dtype=np.float32)
    tgt = np.asarray(tgt).astype(np.int64)

    in1 = (tgt >= C0) & (tgt < C1)
    in2 = tgt >= C1
    perm = np.concatenate(
        [np.nonzero(in1)[0], np.nonzero(in2)[0], np.nonzero(~(in1 | in2))[0]]
    )
    n1 = int(in1.sum())
    n2 = int(in2.sum())
    t0e = (n1 + 127) // 128
    b2 = n1 // 128
    e2 = (n1 + n2 + 127) // 128 if n2 else b2
    n2t = e2 - b2

    tgt_p = tgt[perm]
    inpq = (inp[perm].T * IN_SCALE).astype(FP8)
    inpT = _tiled(inpq)
    if t0e:
        c = np.zeros((D, max(t0e * 128, 128)), FP8)
        c[:, : t0e * 128] = inpq[:, : t0e * 128]
        inpT1 = _tiled(c)

    w1t0 = _tiled((np.asarray(t0_w1, np.float32).T * W1_SCALE).astype(FP8))
    w1t1 = _tiled((np.asarray(t1_w1, np.float32).T * W1_SCALE).astype(FP8))

    hwT_full = np.zeros((D, HEAD_PAD), FP8)
    hwT_full[:, :HEAD] = (np.asarray(head_w, np.float32).T * W1_SCALE).astype(FP8)
    w2t0_full = (np.asarray(t0_w2, np.float32).T * W_SCALE).astype(FP8)
    w2t1_full = np.zeros((H1, T1_PAD), FP8)
    w2t1_full[:, :T1] = (np.asarray(t1_w2, np.float32).T * W_SCALE).astype(FP8)

    gi = np.where(tgt_p < C0, tgt_p, np.where(tgt_p < C1, C0, C0 + 1))
    rel0 = np.where(
        (tgt_p >= C0) & (tgt_p < C1), tgt_p - C0, np.int64(NOREL)
    )[: t0e * 128]
    rel1 = np.where(tgt_p >= C1, tgt_p - C1, np.int64(NOREL))[
        b2 * 128 : e2 * 128
    ]

    # host-gathered target weight rows (bf16, matching device operand
    # precision), zeroed on cores that don't own the target's column shard
    t0_w2_bf = np.asarray(t0_w2, np.float32).astype(BF16)
    t1_w2_bf = np.asarray(t1_w2, np.float32).astype(BF16)

    def _gather_rows(tbl, row, own, nt):
        g = tbl[np.clip(row, 0, tbl.shape[0] - 1)]
        g[~own] = 0
        return np.ascontiguousarray(
            g.reshape(nt, 128, tbl.shape[1]).transpose(1, 0, 2)
        )

    in_maps = []
    for i in range(NCORES):
        m = {
            "inpT": inpT,
            **({"inpT1": inpT1} if t0e else {}),
            "hwT": _tiled(hwT_full[:, i * WH : (i + 1) * WH]),
            "w1t0": w1t0,
            "w1t1": w1t1,
            "w2t0": _tiled(w2t0_full[:, i * W0 : (i + 1) * W0]),
            "w2t1": _tiled(w2t1_full[:, i * W1 : (i + 1) * W1]),
            "misc": _pm((gi - i * WH).astype(np.float64), MT),
        }
        if t0e:
            m["wg0"] = _gather_rows(
                t0_w2_bf, rel0, ((rel0 // W0) == i) & (rel0 >= 0), t0e
            )
        if n2t:
            m["wg1"] = _gather_rows(
                t1_w2_bf, rel1, ((rel1 // W1) == i) & (rel1 >= 0), n2t
            )
        in_maps.append(m)
    return in_maps, perm, n1, n2, t0e, b2, e2


def combine(results, perm, n1, n2, t0e, b2, e2):
    """per-core {'res': [128, NM, 4]} partials -> final [N] f32 NLL."""
    n2t = e2 - b2
    NM = MT + t0e + n2t
    acc = np.zeros((128, NM, 4), np.float64)
    for r in results:
        acc += np.asarray(r["res"], np.float64)

    def col(c):  # [128, NM] -> [NM*128] with row = m*128+p
        return acc[:, :, c].T.reshape(-1)

    S, T, SB = col(0), col(1), col(2)

    S_head = S[:N] - PAD_H  # zero-padded cols contribute exp(0)=1 (core 7)
    out = T[:N] * HID_DESCALE - np.log(S_head)

    if t0e:
        s = slice(N, N + t0e * 128)
        lp0 = T[s] - np.log(S[s] + SB[s])
        out[:n1] += lp0[:n1]
    if n2t:
        s = slice(N + t0e * 128, N + (t0e + n2t) * 128)
        lp1 = T[s] - np.log(S[s] + SB[s] - PAD_1)
        lo = b2 * 128
        out[n1 : n1 + n2] += lp1[n1 - lo : n1 - lo + n2]

    res = np.empty(N, np.float32)
    res[perm] = (-out).astype(np.float32)
    return res


def kernel(inp, tgt, head_w, t0_w1, t0_w2, t1_w1, t1_w2):
    global LAST_RESULT
    in_maps, perm, n1, n2, t0e, b2, e2 = make_in_maps(
        inp, tgt, head_w, t0_w1, t0_w2, t1_w1, t1_w2
    )
    nc = _get_nc(t0e, b2, e2)
    out = run_bass_kernel_spmd(
        nc, in_maps, core_ids=list(range(NCORES)), trace=TRACE
    )
    LAST_RESULT = out
    return combine(out.results, perm, n1, n2, t0e, b2, e2)


# revision 24
# speedup vs baseline: 1.0718x; 1.0515x over previous
"""Trainium2 Bass kernel for AdaptiveLogSoftmaxWithLoss (moe_routing).

Sharding: the three class dimensions are zero-padded and tensor-sharded
across the 8 cores (head 4002->4096, tail0 16000 exact, tail1 30257->30720),
so every core runs an identical SPMD program with 1/8 of the output classes.

Row permutation: the reference discards a tail cluster's logsumexp for rows
whose target is not in that cluster, so the host permutes the sample axis to
[cluster1 rows | cluster2 rows | shortlist rows].  The head runs over all 16
sample tiles (order is irrelevant per-row); the tail pipelines only run over
the ~6 tiles holding cluster-1 rows and the ~10 tiles holding cluster-2 rows
(boundary tiles compute a few wasted rows that the host ignores).  This cuts
exp work ~40% and tail GEMM work ~45%, and the tails read the (permuted)
inpT directly - no compacted input copies.

Per core:
  - hidden projections first (their DMA lands earliest; h0T/h1T in bf16 for
    the XBAR target-dot transposes, plus fp8 copies as tail GEMM lhsT),
  - head logits (fp8 DoubleRow GEMMs, inp 16x / weights 64x) with tail0
    groups interleaved so ACT (the exp bottleneck) never starves: each
    PSUM group gets one ACT exp (+accum_out row sum-exp, descaled via the
    activation scale); logits are small (|x| < ~4) so no max subtraction,
  - head target logit via a fused DVE (iota==rel)*logit pass on the f32
    PSUM group (iota generated once on the idle GpSimd); tail target logits
    dot bf16 natural-layout hidden rows against host-gathered target weight
    rows zeroed on non-owner cores (all-bf16 streams hit the 2x DVE mode -
    an all-fp16 iota==rel pass on the exp output measured 1x and made DVE
    the bottleneck),
  - tail0 tiles split into two 2-bank PSUM groups (1024+976) so the head's
    1-bank pool and tail0's pool coexist; tail1 splits 3840 into a 2048-col
    A group (ACT accum) and 1792-col B group (DVE reduce).

Host combine: sum partials over cores, subtract the exact exp(0)=1
contribution of the zero-padded columns, lse = log(sum), then
NLL = -(head + masked tail terms) scattered back through the permutation.
"""

import numpy as np
import ml_dtypes

import concourse.bass as bass
import concourse.bacc as bacc
import concourse.mybir as mybir
import concourse.tile as tile
from concourse.bass_utils import run_bass_kernel_spmd

BF16 = ml_dtypes.bfloat16
FP8 = ml_dtypes.float8_e4m3
H_SCALE = 8.0     # h cast to fp8 at 8x
W_SCALE = 64.0    # tail w2 cast to fp8 at 64x
IN_SCALE = 16.0   # inp cast to fp8 at 16x
W1_SCALE = 64.0   # w1 / head_w cast to fp8 at 64x
HID_DESCALE = 1.0 / (IN_SCALE * W1_SCALE)
NCORES = 8
N, D = 2048, 1024
H0, H1 = 512, 256
C0, C1 = 4000, 20000
HEAD = 4002        # 4000 shortlist + 2 cluster-logit columns
HEAD_PAD = 4096    # padded so 8 cores get 512 each
T0 = 16000         # divides by 8 exactly (2000 each, no padding)
T1 = 30257
T1_PAD = 30720     # padded so 8 cores get 3840 each
WH, W0, W1 = HEAD_PAD // 8, T0 // 8, T1_PAD // 8     # 512, 2000, 3840
MT = N // 128                                        # 16 sample tiles
PAD_H = HEAD_PAD - HEAD   # 94 zero columns, all on core 7
PAD_1 = T1_PAD - T1       # 463 zero columns, all on core 7
NOREL = -1.0e9            # sentinel rel for rows outside the cluster

# module-level knobs for test.py (harness never touches these)
TRACE = False
LAST_RESULT = None

_CACHED_NC = {}


def _chunks(total, step=512):
    out, o = [], 0
    while o < total:
        out.append((o, min(step, total - o)))
        o += step
    return out


def _build_nc(t0e, b2, e2):
    # tail0 runs on sample tiles [0, t0e); tail1 on [b2, e2)
    n2t = e2 - b2
    n0p, n2p = t0e * 128, n2t * 128
    NM = MT + t0e + n2t
    nc = bacc.Bacc(None)
    BF = mybir.dt.bfloat16
    F8 = mybir.dt.float8e4
    F32 = mybir.dt.float32
    AX = mybir.AxisListType
    OP = mybir.AluOpType
    ACTF = mybir.ActivationFunctionType

    inpT_d = nc.dram_tensor("inpT", [128, D // 128, N], F8, kind="ExternalInput")
    inpT1_d = nc.dram_tensor("inpT1", [128, D // 128, max(n0p, 128)], F8,
                             kind="ExternalInput")
    hwT_d = nc.dram_tensor("hwT", [128, D // 128, WH], F8, kind="ExternalInput")
    w1t0_d = nc.dram_tensor("w1t0", [128, D // 128, H0], F8, kind="ExternalInput")
    w1t1_d = nc.dram_tensor("w1t1", [128, D // 128, H1], F8, kind="ExternalInput")
    w2t0_d = nc.dram_tensor("w2t0", [128, H0 // 128, W0], F8, kind="ExternalInput")
    w2t1_d = nc.dram_tensor("w2t1", [128, H1 // 128, W1], F8, kind="ExternalInput")
    wg0_d = nc.dram_tensor("wg0", [128, max(t0e, 1), H0], BF, kind="ExternalInput")
    wg1_d = nc.dram_tensor("wg1", [128, max(n2t, 1), H1], BF, kind="ExternalInput")
    misc_d = nc.dram_tensor("misc", [128, MT], F32, kind="ExternalInput")
    scr_d = nc.dram_tensor("scr", [128, 512], F8, kind="ExternalOutput")
    res_d = nc.dram_tensor("res", [128, NM, 4], F32, kind="ExternalOutput")

    with tile.TileContext(nc) as tc:
        with (
            tc.tile_pool(name="const", bufs=1) as cp,
            tc.tile_pool(name="work", bufs=3) as wp,
        ):
            inpT = cp.tile([128, D // 128, N], F8)
            inpT1 = cp.tile([128, D // 128, max(n0p, 128)], F8)
            hwT = cp.tile([128, D // 128, WH], F8)
            w1t0 = cp.tile([128, D // 128, H0], F8)
            w1t1 = cp.tile([128, D // 128, H1], F8)
            w2t0 = cp.tile([128, H0 // 128, W0], F8)
            w2t1 = cp.tile([128, H1 // 128, W1], F8)
            wg0 = cp.tile([128, max(t0e, 1), H0], BF)
            wg1 = cp.tile([128, max(n2t, 1), H1], BF)
            relH = cp.tile([128, MT], F32)
            iota = cp.tile([128, WH], F32)
            h0T = cp.tile([128, H0 // 128, max(n0p, 128)], BF)
            h1T = cp.tile([128, H1 // 128, max(n2p, 128)], BF)
            h0T8 = cp.tile([128, H0 // 128, max(n0p, 128)], F8)
            h1T8 = cp.tile([128, H1 // 128, max(n2p, 128)], F8)
            h0n = cp.tile([128, max(t0e, 1), H0], BF)
            h1n = cp.tile([128, max(n2t, 1), H1], BF)
            res = cp.tile([128, NM, 4], F32)

            zz = wp.tile([128, 2], F32, tag="zz")
            nc.vector.memset(zz[:, 0:1], 0)
            nc.scalar.activation(zz[:, 1:2], zz[:, 0:1], ACTF.Exp)
            nc.gpsimd.memset(res[:], 0)
            nc.gpsimd.iota(
                iota[:],
                pattern=[[1, WH]],
                base=0,
                channel_multiplier=0,
                allow_small_or_imprecise_dtypes=True,
            )

            # loads staged in waves: the SDMA engines round-robin across
            # queued transfers, so issuing everything at once starves the
            # critical early tensors.  Later waves sit behind a dummy store
            # whose data dep releases them once hidden0 has progressed.
            nc.sync.dma_start(w1t0[:], w1t0_d[:])
            if t0e:
                nc.sync.dma_start(inpT1[:], inpT1_d[:])

            # head + hidden use 4 one-bank slots, tail0 halves 2 two-bank
            # slots (8 banks total); both close before tail1's 2 four-bank
            # slots open (the boundary is data-serialized on h1T8).
            fpool_cm = tc.tile_pool(name="psumF", bufs=4, space="PSUM")
            fpool = fpool_cm.__enter__()
            mpool_cm = tc.tile_pool(name="psumM", bufs=2, space="PSUM")
            mpool = mpool_cm.__enter__()
            psp = None

            def fslot(w):
                ps = fpool.tile([128, 512], F32, tag="front", name="ps")
                return ps[:, :w]

            def mslot(w):
                ps = mpool.tile([128, 1024], F32, tag="mid", name="ps")
                return ps[:, :w]

            def pslot(w):
                ps = psp.tile([128, 2048], F32, tag="logits", name="ps")
                return ps[:, :w]

            DESCALE = 1.0 / (H_SCALE * W_SCALE)
            DR = mybir.MatmulPerfMode.DoubleRow

            def hidden_block(hT, hT8, w1, inT, coff, npad, mh, alloc):
                # one h k-tile: [128 h, npad samples] in <=512-col chunks
                for co, cw in _chunks(npad):
                    ps = alloc(cw)
                    for kt in range(0, D // 128, 2):
                        nc.tensor.matmul(
                            ps[:],
                            w1[:, kt : kt + 2, mh * 128 : (mh + 1) * 128],
                            inT[:, kt : kt + 2, coff + co : coff + co + cw],
                            start=(kt == 0),
                            stop=(kt + 2 >= D // 128),
                            perf_mode=DR,
                        )
                    nc.vector.tensor_scalar_mul(
                        hT[:, mh, co : co + cw], ps[:], HID_DESCALE
                    )
                    nc.vector.tensor_scalar_mul(
                        hT8[:, mh, co : co + cw], hT[:, mh, co : co + cw],
                        H_SCALE,
                    )

            def head_group(m):
                ms = slice(m * 128, (m + 1) * 128)
                ps = fslot(WH)
                for kt in range(0, D // 128, 2):
                    nc.tensor.matmul(
                        ps[:],
                        inpT[:, kt : kt + 2, ms],
                        hwT[:, kt : kt + 2, :],
                        start=(kt == 0),
                        stop=(kt + 2 >= D // 128),
                        perf_mode=DR,
                    )
                sc_e = wp.tile([128, 2048], BF, tag="sc_e")
                nc.scalar.activation(
                    sc_e[:, :WH],
                    ps[:],
                    ACTF.Exp,
                    scale=HID_DESCALE,
                    accum_out=res[:, m, 0:1],
                )
                sc_t = wp.tile([128, WH], BF, tag="sc_td")
                nc.vector.scalar_tensor_tensor(
                    out=sc_t[:],
                    in0=iota[:],
                    scalar=relH[:, m : m + 1],
                    in1=ps[:],
                    op0=OP.is_equal,
                    op1=OP.mult,
                    accum_out=res[:, m, 1:2],
                )

            def tail_group(lhsT, w2, kdim, mt, gw, goff, s_ap, alloc):
                # fp8 DoubleRow GEMM group + exp/accum partial sum
                ms = slice(mt * 128, (mt + 1) * 128)
                ps = alloc(gw)
                nsub = kdim // 128
                for co, cw in _chunks(gw):
                    for kt in range(0, nsub, 2):
                        nc.tensor.matmul(
                            ps[:, co : co + cw],
                            lhsT[:, kt : kt + 2, ms],
                            w2[:, kt : kt + 2, goff + co : goff + co + cw],
                            start=(kt == 0),
                            stop=(kt + 2 >= nsub),
                            perf_mode=DR,
                        )
                sc_e = wp.tile([128, 2048], BF, tag="sc_e")
                nc.scalar.activation(
                    sc_e[:, :gw], ps[:], ACTF.Exp, scale=DESCALE,
                    accum_out=s_ap,
                )
                return sc_e

            def transposes(hT, hn, hdim):
                # batched XBAR transpose hT[h, r] -> hn[r, h]:
                # out[p, j, q] = in[q, j*128+p]
                for kt in range(hdim // 128):
                    nc.sync.dma_start_transpose(
                        hn[:, :, kt * 128 : (kt + 1) * 128], hT[:, kt, :]
                    )

            def dot(hn, wg, hdim, mt, t_ap):
                sc_d = wp.tile([128, H0], BF, tag="sc_td")
                nc.vector.scalar_tensor_tensor(
                    out=sc_d[:, :hdim],
                    in0=hn[:, mt, :],
                    scalar=1.0,
                    in1=wg[:, mt, :],
                    op0=OP.mult,
                    op1=OP.mult,
                    accum_out=t_ap,
                )

            def tail0_tile(mt):
                tail_group(h0T8, w2t0, H0, mt, 1024, 0,
                           res[:, MT + mt, 0:1], mslot)
                tail_group(h0T8, w2t0, H0, mt, W0 - 1024, 1024,
                           res[:, MT + mt, 2:3], mslot)
                dot(h0n, wg0, H0, mt, res[:, MT + mt, 1:2])

            # emission: hidden0 -> head m0,m1 -> {tail0 t, head m} pairs ->
            # hidden1 woven into the remaining head groups -> tail1
            if True:
                if t0e:
                    hidden_block(h0T, h0T8, w1t0, inpT1, 0, n0p, 0, fslot)
                    nc.sync.dma_start(scr_d[:, 0:128], h0T8[:, 0, 0:128])
                nc.sync.dma_start(hwT[:], hwT_d[:])
                nc.sync.dma_start(inpT[:, :, 0:1024], inpT_d[:, :, 0:1024])
                if t0e:
                    hidden_block(h0T, h0T8, w1t0, inpT1, 0, n0p, 1, fslot)
                    nc.sync.dma_start(scr_d[:, 128:256], h0T8[:, 1, 0:128])
                nc.sync.dma_start(inpT[:, :, 1024:2048], inpT_d[:, :, 1024:2048])
                nc.sync.dma_start(relH[:], misc_d[:])
                if t0e:
                    for mh in range(2, H0 // 128):
                        hidden_block(h0T, h0T8, w1t0, inpT1, 0, n0p, mh, fslot)
                    nc.sync.dma_start(scr_d[:, 384:512], h0T8[:, 3, 0:128])
                nc.sync.dma_start(w2t0[:], w2t0_d[:])
                if t0e:
                    nc.sync.dma_start(wg0[:], wg0_d[:])
                nc.sync.dma_start(w1t1[:], w1t1_d[:])
                if t0e:
                    transposes(h0T, h0n, H0)

            if True:
                m = 0
                while m < min(6, MT):
                    head_group(m)
                    m += 1
                for mt in range(max(t0e - 1, 0)):
                    tail0_tile(mt)
                    if m < MT:
                        head_group(m)
                        m += 1
                h1_at = {max(m, MT - 5): 0, MT - 3: 1} if n2t else {}
                while m < MT:
                    if m in h1_at:
                        hidden_block(
                            h1T, h1T8, w1t1, inpT, b2 * 128, n2p, h1_at[m],
                            fslot,
                        )
                    head_group(m)
                    m += 1
                if t0e:
                    tail0_tile(t0e - 1)
                nc.sync.dma_start(res_d[:, 0:MT], res[:, 0:MT])
                if t0e:
                    nc.sync.dma_start(
                        res_d[:, MT : MT + t0e], res[:, MT : MT + t0e]
                    )
                if n2t:
                    # ensure both hidden1 k-tiles were emitted
                    emitted = {h1_at[k] for k in h1_at if k < MT}
                    for mh in (0, 1):
                        if mh not in emitted:
                            hidden_block(
                                h1T, h1T8, w1t1, inpT, b2 * 128, n2p, mh,
                                fslot,
                            )
                    nc.sync.dma_start(scr_d[:, 256:384], h1T8[:, 1, 0:128])
                nc.sync.dma_start(w2t1[:], w2t1_d[:])
                if n2t:
                    nc.sync.dma_start(wg1[:], wg1_d[:])
                    transposes(h1T, h1n, H1)
            mpool_cm.__exit__(None, None, None)
            fpool_cm.__exit__(None, None, None)
            psp_cm = tc.tile_pool(name="psum", bufs=2, space="PSUM")
            psp = psp_cm.__enter__()
            if True:
                BW = W1 - 2048  # 1792-wide B group
                for mt in range(n2t):
                    ri = MT + t0e + mt
                    dot(h1n, wg1, H1, mt, res[:, ri, 1:2])
                    # B group first: exp on ACT without accum, sum on DVE,
                    # so the final ACT exp (A group) overlaps the B reduce
                    ms = slice(mt * 128, (mt + 1) * 128)
                    ps = pslot(BW)
                    for co, cw in _chunks(BW):
                        nc.tensor.matmul(
                            ps[:, co : co + cw],
                            h1T8[:, 0:2, ms],
                            w2t1[:, 0:2, 2048 + co : 2048 + co + cw],
                            start=True,
                            stop=True,
                            perf_mode=DR,
                        )
                    sc_b = wp.tile([128, 2048], BF, tag="sc_e")
                    nc.scalar.activation(
                        sc_b[:, :BW], ps[:], ACTF.Exp, scale=DESCALE
                    )
                    nc.vector.reduce_sum(res[:, ri, 2:3], sc_b[:, :BW], axis=AX.X)
                    tail_group(h1T8, w2t1, H1, mt, 2048, 0,
                               res[:, ri, 0:1], pslot)

            psp_cm.__exit__(None, None, None)
            if n2t:
                nc.sync.dma_start(
                    res_d[:, MT + t0e :], res[:, MT + t0e :]
                )

    nc.finalize()
    return nc


def _get_nc(t0e, b2, e2):
    key = (t0e, b2, e2)
    if key not in _CACHED_NC:
        _CACHED_NC[key] = _build_nc(t0e, b2, e2)
    return _CACHED_NC[key]


def _tiled(a2d):
    """[K, F] (K multiple of 128) -> contiguous [128, K//128, F]."""
    K, F = a2d.shape
    return np.ascontiguousarray(
        a2d.reshape(K // 128, 128, F).transpose(1, 0, 2)
    )


def _pm(vec, nt):
    """[nt*128] -> [128, nt] with [p, m] = vec[m*128+p]."""
    return np.ascontiguousarray(vec.reshape(nt, 128).T.astype(np.float32))


def make_in_maps(inp, tgt, head_w, t0_w1, t0_w2, t1_w1, t1_w2):
    inp = np.asarray(inp, dtype=np.float32)
    tgt = np.asarray(tgt).astype(np.int64)

    in1 = (tgt >= C0) & (tgt < C1)
    in2 = tgt >= C1
    perm = np.concatenate(
        [np.nonzero(in1)[0], np.nonzero(in2)[0], np.nonzero(~(in1 | in2))[0]]
    )
    n1 = int(in1.sum())
    n2 = int(in2.sum())
    t0e = (n1 + 127) // 128
    b2 = n1 // 128
    e2 = (n1 + n2 + 127) // 128 if n2 else b2
    n2t = e2 - b2

    tgt_p = tgt[perm]
    inpq = (inp[perm].T * IN_SCALE).astype(FP8)
    inpT = _tiled(inpq)
    if t0e:
        c = np.zeros((D, max(t0e * 128, 128)), FP8)
        c[:, : t0e * 128] = inpq[:, : t0e * 128]
        inpT1 = _tiled(c)

    w1t0 = _tiled((np.asarray(t0_w1, np.float32).T * W1_SCALE).astype(FP8))
    w1t1 = _tiled((np.asarray(t1_w1, np.float32).T * W1_SCALE).astype(FP8))

    hwT_full = np.zeros((D, HEAD_PAD), FP8)
    hwT_full[:, :HEAD] = (np.asarray(head_w, np.float32).T * W1_SCALE).astype(FP8)
    w2t0_full = (np.asarray(t0_w2, np.float32).T * W_SCALE).astype(FP8)
    w2t1_full = np.zeros((H1, T1_PAD), FP8)
    w2t1_full[:, :T1] = (np.asarray(t1_w2, np.float32).T * W_SCALE).astype(FP8)

    gi = np.where(tgt_p < C0, tgt_p, np.where(tgt_p < C1, C0, C0 + 1))
    rel0 = np.where(
        (tgt_p >= C0) & (tgt_p < C1), tgt_p - C0, np.int64(NOREL)
    )[: t0e * 128]
    rel1 = np.where(tgt_p >= C1, tgt_p - C1, np.int64(NOREL))[
        b2 * 128 : e2 * 128
    ]

    # host-gathered target weight rows (bf16, matching device operand
    # precision), zeroed on cores that don't own the target's column shard
    t0_w2_bf = np.asarray(t0_w2, np.float32).astype(BF16)
    t1_w2_bf = np.asarray(t1_w2, np.float32).astype(BF16)

    def _gather_rows(tbl, row, own, nt):
        g = tbl[np.clip(row, 0, tbl.shape[0] - 1)]
        g[~own] = 0
        return np.ascontiguousarray(
            g.reshape(nt, 128, tbl.shape[1]).transpose(1, 0, 2)
        )

    in_maps = []
    for i in range(NCORES):
        m = {
            "inpT": inpT,
            **({"inpT1": inpT1} if t0e else {}),
            "hwT": _tiled(hwT_full[:, i * WH : (i + 1) * WH]),
            "w1t0": w1t0,
            "w1t1": w1t1,
            "w2t0": _tiled(w2t0_full[:, i * W0 : (i + 1) * W0]),
            "w2t1": _tiled(w2t1_full[:, i * W1 : (i + 1) * W1]),
            "misc": _pm((gi - i * WH).astype(np.float64), MT),
        }
        if t0e:
            m["wg0"] = _gather_rows(
                t0_w2_bf, rel0, ((rel0 // W0) == i) & (rel0 >= 0), t0e
            )
        if n2t:
            m["wg1"] = _gather_rows(
                t1_w2_bf, rel1, ((rel1 // W1) == i) & (rel1 >= 0), n2t
            )
        in_maps.append(m)
    return in_maps, perm, n1, n2, t0e, b2, e2


def combine(results, perm, n1, n2, t0e, b2, e2):
    """per-core {'res': [128, NM, 4]} partials -> final [N] f32 NLL."""
    n2t = e2 - b2
    NM = MT + t0e + n2t
    acc = np.zeros((128, NM, 4), np.float64)
    for r in results:
        acc += np.asarray(r["res"], np.float64)

    def col(c):  # [128, NM] -> [NM*128] with row = m*128+p
        return acc[:, :, c].T.reshape(-1)

    S, T, SB = col(0), col(1), col(2)

    S_head = S[:N] - PAD_H  # zero-padded cols contribute exp(0)=1 (core 7)
    out = T[:N] * HID_DESCALE - np.log(S_head)

    if t0e:
        s = slice(N, N + t0e * 128)
        lp0 = T[s] - np.log(S[s] + SB[s])
        out[:n1] += lp0[:n1]
    if n2t:
        s = slice(N + t0e * 128, N + (t0e + n2t) * 128)
        lp1 = T[s] - np.log(S[s] + SB[s] - PAD_1)
        lo = b2 * 128
        out[n1 : n1 + n2] += lp1[n1 - lo : n1 - lo + n2]

    res = np.empty(N, np.float32)
    res[perm] = (-out).astype(np.float32)
    return res


def kernel(inp, tgt, head_w, t0_w1, t0_w2, t1_w1, t1_w2):
    global LAST_RESULT
    in_maps, perm, n1, n2, t0e, b2, e2 = make_in_maps(
        inp, tgt, head_w, t0_w1, t0_w2, t1_w1, t1_w2
    )
    nc = _get_nc(t0e, b2, e2)
    out = run_bass_kernel_spmd(
        nc, in_maps, core_ids=list(range(NCORES)), trace=TRACE
    )
    LAST_RESULT = out
    return combine(out.results, perm, n1, n2, t0e, b2, e2)
